# revision 11
# baseline (speedup 1.0000x reference)
"""Trainium2 Bass kernel for the 4-layer sum/product circuit
(nn_KnowledgeLayer): h = enc(x); h = h[idx0].prod(1); h = h[idx1].sum(1);
h = h[idx2].prod(1); h = h[idx3].sum(1).

Strategy (slot-sharded + edge-fused gather, v5):
  * Host composes the four index maps into per-output factor pairs
    (A,B) of table rows (8 h0-slots per final output, depth-first).
  * Host lays the per-core table out as a trail cover of the distinct
    {A,B} pair multigraph (Hierholzer walks): every distinct factor
    pair appears as two CONSECUTIVE rows somewhere in the sequence.
    One SWDGE gather descriptor of 2 rows (4 KB bf16) then fetches BOTH
    factors of a slot: 4096 descriptors per core instead of 16384.
    SWDGE descriptor generation (~8.3 ns/desc on the Q7) was the
    bottleneck; the gather is now HBM-bandwidth-bound.
  * Each of the 8 cores computes 512 of the 4096 output rows over the
    full 1024 batch; DVE tree-reduces mul/add/mul/add in bf16 (f32
    final) and DMAs its [512, 1024] slice out.  Outputs concatenate on
    axis 0.
"""

import numpy as np

N_VARS = 2048
BATCH = 1024
NCORES = 8
TABLE_ROWS = 2 * N_VARS + 2      # 4098 logical encode rows
NOUT = 4096                      # h3 rows total
CORE_OUT = NOUT // NCORES        # 512 output rows per core
NCHUNK = CORE_OUT // 128         # 4 chunks of 128 outputs
GI = 1024                        # slots per gather call (128 outs x 8)
LMAX = 8192                      # padded trail-table rows (worst case 2*4096)


# ----------------------------------------------------------------------------
# NTFF profile hook shim: this image's `antenv` lacks `axon_hooks`, which
# bass_utils imports unconditionally when trace=True under axon.  Provide the
# module and register the ctypes-based hook from trn_agent_boot.
# ----------------------------------------------------------------------------

def _ensure_ntff_hook():
    import sys
    try:
        from antenv import axon_hooks  # noqa: F401
        return
    except ImportError:
        pass
    import types
    mod = types.ModuleType("antenv.axon_hooks")
    mod._hook = None

    def set_axon_ntff_profile_hook(h):
        mod._hook = h

    def get_axon_ntff_profile_hook():
        return mod._hook

    mod.set_axon_ntff_profile_hook = set_axon_ntff_profile_hook
    mod.get_axon_ntff_profile_hook = get_axon_ntff_profile_hook
    sys.modules["antenv.axon_hooks"] = mod
    try:
        import antenv
        antenv.axon_hooks = mod
    except ImportError:
        pass
    try:
        from trn_agent_boot.trn_boot import _ntff_profile_via_ctypes
        hook = _ntff_profile_via_ctypes("/opt/axon/libaxon_pjrt.so")
        if hook is not None:
            mod._hook = hook
    except Exception:
        pass


try:
    _ensure_ntff_hook()
except Exception:
    pass


# ----------------------------------------------------------------------------
# host-side index preparation
# ----------------------------------------------------------------------------

def _compose_indices(idx0, idx1, idx2, idx3):
    """Return S_A, S_B: [4096, 8] table indices (slot k of output o)."""
    J = idx3.reshape(-1)
    K = idx2[J].reshape(-1)
    L = idx1[K].reshape(-1)
    AB = idx0[L]
    A, B = AB[:, 0].astype(np.int64), AB[:, 1].astype(np.int64)

    def remap(e):
        out = np.empty_like(e)
        out[e == 0] = 2 * N_VARS
        out[e == 1] = 2 * N_VARS + 1
        even = (e >= 2) & (e % 2 == 0)
        out[even] = (e[even] - 2) // 2
        odd = (e >= 3) & (e % 2 == 1)
        out[odd] = N_VARS + (e[odd] - 3) // 2
        return out

    return remap(A).reshape(NOUT, 8), remap(B).reshape(NOUT, 8)


def _build_trail(SA, SB, c):
    """Trail-cover table for core c.

    Returns (seq, pos) where seq is the row sequence (list of logical
    table rows, len <= LMAX) and pos[s] (s in [0,4096)) is the position
    of slot s's factor pair: seq[pos[s]] and seq[pos[s]+1] are the two
    factors of slot s.
    """
    A = SA[c * CORE_OUT:(c + 1) * CORE_OUT].reshape(-1)
    B = SB[c * CORE_OUT:(c + 1) * CORE_OUT].reshape(-1)

    from collections import defaultdict
    edge_pos = {}
    adj = defaultdict(list)       # node -> list of edge keys
    loops = []
    keys = []
    for a, b in zip(A.tolist(), B.tolist()):
        k = (a, b) if a <= b else (b, a)
        keys.append(k)
        if k not in edge_pos:
            edge_pos[k] = -1
            if a == b:
                loops.append(k)
            else:
                adj[k[0]].append(k)
                adj[k[1]].append(k)

    deg = {u: len(es) for u, es in adj.items()}
    used = set()
    seq = []

    def walk(start):
        # greedy trail walk, no splicing: consecutive nodes are always
        # genuine edges; leftover edges get their own trails later
        trail = [start]
        u = start
        while True:
            found = None
            while adj[u]:
                k = adj[u].pop()
                if k in used:
                    continue
                used.add(k)
                found = k
                break
            if found is None:
                break
            v = found[0] if found[1] == u else found[1]
            trail.append(v)
            u = v
        return trail

    # prefer odd-degree starts so each component needs fewest trails
    nodes = sorted(deg, key=lambda u: -(deg[u] % 2))
    for u in nodes:
        while any(k not in used for k in adj[u]):
            trail = walk(u)
            base = len(seq)
            seq.extend(trail)
            for i in range(len(trail) - 1):
                a, b = trail[i], trail[i + 1]
                k = (a, b) if a <= b else (b, a)
                if edge_pos[k] == -1:
                    edge_pos[k] = base + i
    for k in loops:
        edge_pos[k] = len(seq)
        seq.extend([k[0], k[0]])

    pos = np.array([edge_pos[k] for k in keys], dtype=np.int64)
    assert len(seq) <= LMAX, len(seq)
    return seq, pos


# per-chunk gather call plan: list of block spans [k0, k1) per chunk.
# All calls are 512-idx halves: the first transfer starts after only
# half a call of DGE, and the critical-path tail after the last
# transfer is just half a chunk of DVE (split reduction tree).
CALL_PLAN = [[(0, 4), (4, 8)]] * NCHUNK


def _wrap_core_idx(pos):
    """Wrap per-slot positions [4096] into the SWDGE index tensor
    [128, 4096//16] int16, ordered per CALL_PLAN.  Within a call over
    chunk cc blocks [k0,k1): element i fetches slot
    (cc*128 + i%128)*8 + (k0 + i//128)."""
    S = pos.reshape(CORE_OUT, 8)
    cols = []
    for cc, calls in enumerate(CALL_PLAN):
        blk = S[cc * 128:(cc + 1) * 128, :]      # [128 outs, 8 slots]
        for k0, k1 in calls:
            idx_call = blk[:, k0:k1].T.reshape(-1)
            w = idx_call.reshape(-1, 16).T.astype(np.int16)
            cols.append(np.tile(w, (8, 1)))
    return np.ascontiguousarray(np.concatenate(cols, axis=1))


# ----------------------------------------------------------------------------
# bass program (built once, cached)
# ----------------------------------------------------------------------------

_CACHED = {}


def _build_program():
    import concourse.bacc as bacc
    import concourse.mybir as mybir
    from concourse.tile import TileContext
    from concourse.ap import AP

    f32 = mybir.dt.float32
    bf16 = mybir.dt.bfloat16
    i16 = mybir.dt.int16

    nc = bacc.Bacc("TRN2", target_bir_lowering=False, debug=False)

    enc = nc.dram_tensor("enc", [LMAX, BATCH], bf16, kind="ExternalInput")
    idxe = nc.dram_tensor("idxe", [128, NCHUNK * GI // 16], i16,
                          kind="ExternalInput")
    out = nc.dram_tensor("out", [CORE_OUT, BATCH], f32, kind="ExternalOutput")

    E2 = 2 * BATCH   # gathered element: 2 consecutive rows = both factors

    with TileContext(nc) as tc:
        with tc.tile_pool(name="setup", bufs=1) as sp, \
             tc.tile_pool(name="gather", bufs=4) as gp, \
             tc.tile_pool(name="mid", bufs=3) as mp, \
             tc.tile_pool(name="outp", bufs=2) as op:

            ia = sp.tile([128, NCHUNK * GI // 16], i16, tag="ia")
            nc.sync.dma_start(out=ia[:, :], in_=idxe[:, :])

            # overlapping-window view of enc: element p = rows [p, p+1]
            base = enc[:, :]
            enc_win = AP(tensor=base.tensor, offset=base.offset,
                         ap=[[BATCH, LMAX - 1], [1, E2]])

            icol = 0
            for cc in range(NCHUNK):
                h2s = []
                for h in range(2):
                    g = gp.tile([128, 4, E2], bf16, tag="gh")
                    nc.gpsimd.dma_gather(
                        out_ap=g[:, :, :],
                        in_ap=enc_win,
                        idxs_ap=ia[:, icol:icol + 32],
                        num_idxs=512, num_idxs_reg=512,
                        elem_size=E2, elem_step=BATCH,
                        single_packet=False)
                    icol += 32
                    # block k = [factor_u | factor_v] of slot 4h+k
                    h0 = mp.tile([128, 4, BATCH], bf16, tag="h0h")
                    nc.vector.tensor_mul(
                        h0[:, :, :], g[:, :, 0:BATCH], g[:, :, BATCH:E2])
                    h1 = mp.tile([128, 2, BATCH], bf16, tag="h1h")
                    nc.vector.tensor_add(
                        h1[:, :, :], h0[:, 0:4:2, :], h0[:, 1:4:2, :])
                    h2 = mp.tile([128, 1, BATCH], bf16, tag="h2h")
                    nc.vector.tensor_mul(
                        h2[:, :, :], h1[:, 0:1, :], h1[:, 1:2, :])
                    h2s.append(h2)
                h3 = op.tile([128, 1, BATCH], f32, tag="h3")
                nc.vector.tensor_add(
                    h3[:, :, :], h2s[0][:, :, :], h2s[1][:, :, :])
                nc.sync.dma_start(
                    out=out[cc * 128:(cc + 1) * 128, :]
                        .rearrange("(k p) f -> p k f", p=128),
                    in_=h3[:, :, :])

    nc.compile()
    return nc


def _get_program():
    if "nc" not in _CACHED:
        _CACHED["nc"] = _build_program()
    return _CACHED["nc"]


# ----------------------------------------------------------------------------
# public entry point
# ----------------------------------------------------------------------------

def kernel(x, idx0, idx1, idx2, idx3, _trace=False, _trace_kwargs=None):
    import ml_dtypes
    from concourse.bass_utils import run_bass_kernel_spmd

    x = np.asarray(x, dtype=np.float32)
    enc_np = np.empty((TABLE_ROWS, BATCH), np.float32)
    enc_np[0:N_VARS] = x
    enc_np[N_VARS:2 * N_VARS] = 1.0 - x
    enc_np[2 * N_VARS] = 0.0
    enc_np[2 * N_VARS + 1] = 1.0
    enc_bf = enc_np.astype(ml_dtypes.bfloat16)
    S_A, S_B = _compose_indices(
        np.asarray(idx0), np.asarray(idx1), np.asarray(idx2), np.asarray(idx3))

    nc = _get_program()
    in_maps = []
    for c in range(NCORES):
        seq, pos = _build_trail(S_A, S_B, c)
        table = np.zeros((LMAX, BATCH), ml_dtypes.bfloat16)
        table[:len(seq)] = enc_bf[np.asarray(seq, dtype=np.int64)]
        in_maps.append({
            "enc": table,
            "idxe": _wrap_core_idx(pos),
        })

    kwargs = {}
    if _trace:
        kwargs["trace"] = True
        if _trace_kwargs:
            kwargs.update(_trace_kwargs)
    res = run_bass_kernel_spmd(nc, in_maps, core_ids=list(range(NCORES)), **kwargs)
    outs = [res.results[c]["out"] for c in range(NCORES)]
    full = np.concatenate(outs, axis=0)
    if _trace:
        kernel.last_exec_time_ns = res.exec_time_ns
        kernel.last_profile = res.profile_json
    return full


# revision 13
# speedup vs baseline: 1.0255x; 1.0255x over previous
"""Trainium2 Bass kernel for the 4-layer sum/product circuit
(nn_KnowledgeLayer): h = enc(x); h = h[idx0].prod(1); h = h[idx1].sum(1);
h = h[idx2].prod(1); h = h[idx3].sum(1).

Strategy (slot-sharded + edge-fused gather, v5):
  * Host composes the four index maps into per-output factor pairs
    (A,B) of table rows (8 h0-slots per final output, depth-first).
  * Host lays the per-core table out as a trail cover of the distinct
    {A,B} pair multigraph (Hierholzer walks): every distinct factor
    pair appears as two CONSECUTIVE rows somewhere in the sequence.
    One SWDGE gather descriptor of 2 rows (4 KB bf16) then fetches BOTH
    factors of a slot: 4096 descriptors per core instead of 16384.
    SWDGE descriptor generation (~8.3 ns/desc on the Q7) was the
    bottleneck; the gather is now HBM-bandwidth-bound.
  * Each of the 8 cores computes 512 of the 4096 output rows over the
    full 1024 batch; DVE tree-reduces mul/add/mul/add in bf16 (f32
    final) and DMAs its [512, 1024] slice out.  Outputs concatenate on
    axis 0.
"""

import numpy as np

N_VARS = 2048
BATCH = 1024
NCORES = 8
TABLE_ROWS = 2 * N_VARS + 2      # 4098 logical encode rows
NOUT = 4096                      # h3 rows total
CORE_OUT = NOUT // NCORES        # 512 output rows per core
NCHUNK = CORE_OUT // 128         # 4 chunks of 128 outputs
GI = 1024                        # slots per gather call (128 outs x 8)
LMAX = 8192                      # padded trail-table rows (worst case 2*4096)


# ----------------------------------------------------------------------------
# NTFF profile hook shim: this image's `antenv` lacks `axon_hooks`, which
# bass_utils imports unconditionally when trace=True under axon.  Provide the
# module and register the ctypes-based hook from trn_agent_boot.
# ----------------------------------------------------------------------------

def _ensure_ntff_hook():
    import sys
    try:
        from antenv import axon_hooks  # noqa: F401
        return
    except ImportError:
        pass
    import types
    mod = types.ModuleType("antenv.axon_hooks")
    mod._hook = None

    def set_axon_ntff_profile_hook(h):
        mod._hook = h

    def get_axon_ntff_profile_hook():
        return mod._hook

    mod.set_axon_ntff_profile_hook = set_axon_ntff_profile_hook
    mod.get_axon_ntff_profile_hook = get_axon_ntff_profile_hook
    sys.modules["antenv.axon_hooks"] = mod
    try:
        import antenv
        antenv.axon_hooks = mod
    except ImportError:
        pass
    try:
        from trn_agent_boot.trn_boot import _ntff_profile_via_ctypes
        hook = _ntff_profile_via_ctypes("/opt/axon/libaxon_pjrt.so")
        if hook is not None:
            mod._hook = hook
    except Exception:
        pass


try:
    _ensure_ntff_hook()
except Exception:
    pass


# ----------------------------------------------------------------------------
# host-side index preparation
# ----------------------------------------------------------------------------

def _compose_indices(idx0, idx1, idx2, idx3):
    """Return S_A, S_B: [4096, 8] table indices (slot k of output o)."""
    J = idx3.reshape(-1)
    K = idx2[J].reshape(-1)
    L = idx1[K].reshape(-1)
    AB = idx0[L]
    A, B = AB[:, 0].astype(np.int64), AB[:, 1].astype(np.int64)

    def remap(e):
        out = np.empty_like(e)
        out[e == 0] = 2 * N_VARS
        out[e == 1] = 2 * N_VARS + 1
        even = (e >= 2) & (e % 2 == 0)
        out[even] = (e[even] - 2) // 2
        odd = (e >= 3) & (e % 2 == 1)
        out[odd] = N_VARS + (e[odd] - 3) // 2
        return out

    return remap(A).reshape(NOUT, 8), remap(B).reshape(NOUT, 8)


def _build_trail(SA, SB, c):
    """Trail-cover table for core c.

    Returns (seq, pos) where seq is the row sequence (list of logical
    table rows, len <= LMAX) and pos[s] (s in [0,4096)) is the position
    of slot s's factor pair: seq[pos[s]] and seq[pos[s]+1] are the two
    factors of slot s.
    """
    A = SA[c * CORE_OUT:(c + 1) * CORE_OUT].reshape(-1)
    B = SB[c * CORE_OUT:(c + 1) * CORE_OUT].reshape(-1)

    from collections import defaultdict
    edge_pos = {}
    adj = defaultdict(list)       # node -> list of edge keys
    loops = []
    keys = []
    for a, b in zip(A.tolist(), B.tolist()):
        k = (a, b) if a <= b else (b, a)
        keys.append(k)
        if k not in edge_pos:
            edge_pos[k] = -1
            if a == b:
                loops.append(k)
            else:
                adj[k[0]].append(k)
                adj[k[1]].append(k)

    deg = {u: len(es) for u, es in adj.items()}
    used = set()
    seq = []

    def walk(start):
        # greedy trail walk, no splicing: consecutive nodes are always
        # genuine edges; leftover edges get their own trails later
        trail = [start]
        u = start
        while True:
            found = None
            while adj[u]:
                k = adj[u].pop()
                if k in used:
                    continue
                used.add(k)
                found = k
                break
            if found is None:
                break
            v = found[0] if found[1] == u else found[1]
            trail.append(v)
            u = v
        return trail

    # prefer odd-degree starts so each component needs fewest trails
    nodes = sorted(deg, key=lambda u: -(deg[u] % 2))
    for u in nodes:
        while any(k not in used for k in adj[u]):
            trail = walk(u)
            base = len(seq)
            seq.extend(trail)
            for i in range(len(trail) - 1):
                a, b = trail[i], trail[i + 1]
                k = (a, b) if a <= b else (b, a)
                if edge_pos[k] == -1:
                    edge_pos[k] = base + i
    for k in loops:
        edge_pos[k] = len(seq)
        seq.extend([k[0], k[0]])

    pos = np.array([edge_pos[k] for k in keys], dtype=np.int64)
    assert len(seq) <= LMAX, len(seq)
    return seq, pos


# per-chunk gather call plan: list of block spans [k0, k1) per chunk.
# 512-idx halves keep the first transfer early; the last chunk's second
# half splits further (256+256) so the post-transfer DVE tail is tiny.
CALL_PLAN = [[(0, 4), (4, 8)]] * (NCHUNK - 1) + [[(0, 4), (4, 6), (6, 8)]]


def _wrap_core_idx(pos):
    """Wrap per-slot positions [4096] into the SWDGE index tensor
    [128, 4096//16] int16, ordered per CALL_PLAN.  Within a call over
    chunk cc blocks [k0,k1): element i fetches slot
    (cc*128 + i%128)*8 + (k0 + i//128)."""
    S = pos.reshape(CORE_OUT, 8)
    cols = []
    for cc, calls in enumerate(CALL_PLAN):
        blk = S[cc * 128:(cc + 1) * 128, :]      # [128 outs, 8 slots]
        for k0, k1 in calls:
            idx_call = blk[:, k0:k1].T.reshape(-1)
            w = idx_call.reshape(-1, 16).T.astype(np.int16)
            cols.append(np.tile(w, (8, 1)))
    return np.ascontiguousarray(np.concatenate(cols, axis=1))


# ----------------------------------------------------------------------------
# bass program (built once, cached)
# ----------------------------------------------------------------------------

_CACHED = {}


def _build_program():
    import concourse.bacc as bacc
    import concourse.mybir as mybir
    from concourse.tile import TileContext
    from concourse.ap import AP

    f32 = mybir.dt.float32
    bf16 = mybir.dt.bfloat16
    i16 = mybir.dt.int16

    nc = bacc.Bacc("TRN2", target_bir_lowering=False, debug=False)

    enc = nc.dram_tensor("enc", [LMAX, BATCH], bf16, kind="ExternalInput")
    idxe = nc.dram_tensor("idxe", [128, NCHUNK * GI // 16], i16,
                          kind="ExternalInput")
    out = nc.dram_tensor("out", [CORE_OUT, BATCH], f32, kind="ExternalOutput")

    E2 = 2 * BATCH   # gathered element: 2 consecutive rows = both factors

    with TileContext(nc) as tc:
        with tc.tile_pool(name="setup", bufs=1) as sp, \
             tc.tile_pool(name="gather", bufs=7) as gp, \
             tc.tile_pool(name="gatherq", bufs=2) as gq, \
             tc.tile_pool(name="mid", bufs=3) as mp, \
             tc.tile_pool(name="outp", bufs=2) as op:

            ia = sp.tile([128, NCHUNK * GI // 16], i16, tag="ia")
            nc.sync.dma_start(out=ia[:, :], in_=idxe[:, :])

            # overlapping-window view of enc: element p = rows [p, p+1]
            base = enc[:, :]
            enc_win = AP(tensor=base.tensor, offset=base.offset,
                         ap=[[BATCH, LMAX - 1], [1, E2]])

            icol = 0

            def gather(pool, nblk, tag):
                nonlocal icol
                g = pool.tile([128, nblk, E2], bf16, tag=tag)
                nidx = nblk * 128
                nc.gpsimd.dma_gather(
                    out_ap=g[:, :, :],
                    in_ap=enc_win,
                    idxs_ap=ia[:, icol:icol + nidx // 16],
                    num_idxs=nidx, num_idxs_reg=nidx,
                    elem_size=E2, elem_step=BATCH,
                    single_packet=False)
                icol += nidx // 16
                return g

            def half_tree(g):
                # block k of g = [factor_u | factor_v]; returns the
                # 4-slot half-tree result h2 = (s0*s1' ... ) [128,1,B]
                h0 = mp.tile([128, 4, BATCH], bf16, tag="h0h")
                nc.vector.tensor_mul(
                    h0[:, :, :], g[:, :, 0:BATCH], g[:, :, BATCH:E2])
                h1 = mp.tile([128, 2, BATCH], bf16, tag="h1h")
                nc.vector.tensor_add(
                    h1[:, :, :], h0[:, 0:4:2, :], h0[:, 1:4:2, :])
                h2 = mp.tile([128, 1, BATCH], bf16, tag="h2h")
                nc.vector.tensor_mul(
                    h2[:, :, :], h1[:, 0:1, :], h1[:, 1:2, :])
                return h2

            def emit_out(cc, h2a, h2b):
                h3 = op.tile([128, 1, BATCH], f32, tag="h3")
                nc.vector.tensor_add(
                    h3[:, :, :], h2a[:, :, :], h2b[:, :, :])
                nc.sync.dma_start(
                    out=out[cc * 128:(cc + 1) * 128, :]
                        .rearrange("(k p) f -> p k f", p=128),
                    in_=h3[:, :, :])

            # chunks 0-2: two 512-idx halves each
            for cc in range(NCHUNK - 1):
                h2a = half_tree(gather(gp, 4, "gh"))
                h2b = half_tree(gather(gp, 4, "gh"))
                emit_out(cc, h2a, h2b)

            # chunk 3: 512 + 256 + 256 so the post-transfer DVE tail is
            # only the last quarter's ops
            h2a = half_tree(gather(gp, 4, "gh"))
            g1 = gather(gq, 2, "gq")
            g2 = gather(gq, 2, "gq")
            q1 = mp.tile([128, 2, BATCH], bf16, tag="h0q")
            nc.vector.tensor_mul(
                q1[:, :, :], g1[:, :, 0:BATCH], g1[:, :, BATCH:E2])
            p1 = mp.tile([128, 1, BATCH], bf16, tag="h1q")
            nc.vector.tensor_add(p1[:, :, :], q1[:, 0:1, :], q1[:, 1:2, :])
            q2 = mp.tile([128, 2, BATCH], bf16, tag="h0q")
            nc.vector.tensor_mul(
                q2[:, :, :], g2[:, :, 0:BATCH], g2[:, :, BATCH:E2])
            p2 = mp.tile([128, 1, BATCH], bf16, tag="h1q")
            nc.vector.tensor_add(p2[:, :, :], q2[:, 0:1, :], q2[:, 1:2, :])
            h2b = mp.tile([128, 1, BATCH], bf16, tag="h2h")
            nc.vector.tensor_mul(h2b[:, :, :], p1[:, :, :], p2[:, :, :])
            emit_out(NCHUNK - 1, h2a, h2b)

    nc.compile()
    return nc


def _get_program():
    if "nc" not in _CACHED:
        _CACHED["nc"] = _build_program()
    return _CACHED["nc"]


# ----------------------------------------------------------------------------
# public entry point
# ----------------------------------------------------------------------------

def kernel(x, idx0, idx1, idx2, idx3, _trace=False, _trace_kwargs=None):
    import ml_dtypes
    from concourse.bass_utils import run_bass_kernel_spmd

    x = np.asarray(x, dtype=np.float32)
    enc_np = np.empty((TABLE_ROWS, BATCH), np.float32)
    enc_np[0:N_VARS] = x
    enc_np[N_VARS:2 * N_VARS] = 1.0 - x
    enc_np[2 * N_VARS] = 0.0
    enc_np[2 * N_VARS + 1] = 1.0
    enc_bf = enc_np.astype(ml_dtypes.bfloat16)
    S_A, S_B = _compose_indices(
        np.asarray(idx0), np.asarray(idx1), np.asarray(idx2), np.asarray(idx3))

    nc = _get_program()
    in_maps = []
    for c in range(NCORES):
        seq, pos = _build_trail(S_A, S_B, c)
        table = np.zeros((LMAX, BATCH), ml_dtypes.bfloat16)
        table[:len(seq)] = enc_bf[np.asarray(seq, dtype=np.int64)]
        in_maps.append({
            "enc": table,
            "idxe": _wrap_core_idx(pos),
        })

    kwargs = {}
    if _trace:
        kwargs["trace"] = True
        if _trace_kwargs:
            kwargs.update(_trace_kwargs)
    res = run_bass_kernel_spmd(nc, in_maps, core_ids=list(range(NCORES)), **kwargs)
    outs = [res.results[c]["out"] for c in range(NCORES)]
    full = np.concatenate(outs, axis=0)
    if _trace:
        kernel.last_exec_time_ns = res.exec_time_ns
        kernel.last_profile = res.profile_json
    return full


# revision 17
# speedup vs baseline: 1.1135x; 1.0858x over previous
"""Trainium2 Bass kernel for the 4-layer sum/product circuit
(nn_KnowledgeLayer): h = enc(x); h = h[idx0].prod(1); h = h[idx1].sum(1);
h = h[idx2].prod(1); h = h[idx3].sum(1).

Strategy (slot-sharded + edge-fused gather, v5):
  * Host composes the four index maps into per-output factor pairs
    (A,B) of table rows (8 h0-slots per final output, depth-first).
  * Host lays the per-core table out as a trail cover of the distinct
    {A,B} pair multigraph (Hierholzer walks): every distinct factor
    pair appears as two CONSECUTIVE rows somewhere in the sequence.
    One SWDGE gather descriptor of 2 rows (4 KB bf16) then fetches BOTH
    factors of a slot: 4096 descriptors per core instead of 16384.
    SWDGE descriptor generation (~8.3 ns/desc on the Q7) was the
    bottleneck; the gather is now HBM-bandwidth-bound.
  * Each of the 8 cores computes 512 of the 4096 output rows over the
    full 1024 batch; DVE tree-reduces mul/add/mul/add in bf16 (f32
    final) and DMAs its [512, 1024] slice out.  Outputs concatenate on
    axis 0.
"""

import numpy as np

N_VARS = 2048
BATCH = 1024
NCORES = 8
TABLE_ROWS = 2 * N_VARS + 2      # 4098 logical encode rows
NOUT = 4096                      # h3 rows total
CORE_OUT = NOUT // NCORES        # 512 output rows per core
NCHUNK = CORE_OUT // 128         # 4 chunks of 128 outputs
GI = 1024                        # slots per gather call (128 outs x 8)
LMAX = 8192                      # padded trail-table rows (worst case 2*4096)


# ----------------------------------------------------------------------------
# NTFF profile hook shim: this image's `antenv` lacks `axon_hooks`, which
# bass_utils imports unconditionally when trace=True under axon.  Provide the
# module and register the ctypes-based hook from trn_agent_boot.
# ----------------------------------------------------------------------------

def _ensure_ntff_hook():
    import sys
    try:
        from antenv import axon_hooks  # noqa: F401
        return
    except ImportError:
        pass
    import types
    mod = types.ModuleType("antenv.axon_hooks")
    mod._hook = None

    def set_axon_ntff_profile_hook(h):
        mod._hook = h

    def get_axon_ntff_profile_hook():
        return mod._hook

    mod.set_axon_ntff_profile_hook = set_axon_ntff_profile_hook
    mod.get_axon_ntff_profile_hook = get_axon_ntff_profile_hook
    sys.modules["antenv.axon_hooks"] = mod
    try:
        import antenv
        antenv.axon_hooks = mod
    except ImportError:
        pass
    try:
        from trn_agent_boot.trn_boot import _ntff_profile_via_ctypes
        hook = _ntff_profile_via_ctypes("/opt/axon/libaxon_pjrt.so")
        if hook is not None:
            mod._hook = hook
    except Exception:
        pass


try:
    _ensure_ntff_hook()
except Exception:
    pass


# ----------------------------------------------------------------------------
# host-side index preparation
# ----------------------------------------------------------------------------

def _compose_indices(idx0, idx1, idx2, idx3):
    """Return S_A, S_B: [4096, 8] table indices (slot k of output o)."""
    J = idx3.reshape(-1)
    K = idx2[J].reshape(-1)
    L = idx1[K].reshape(-1)
    AB = idx0[L]
    A, B = AB[:, 0].astype(np.int64), AB[:, 1].astype(np.int64)

    def remap(e):
        out = np.empty_like(e)
        out[e == 0] = 2 * N_VARS
        out[e == 1] = 2 * N_VARS + 1
        even = (e >= 2) & (e % 2 == 0)
        out[even] = (e[even] - 2) // 2
        odd = (e >= 3) & (e % 2 == 1)
        out[odd] = N_VARS + (e[odd] - 3) // 2
        return out

    return remap(A).reshape(NOUT, 8), remap(B).reshape(NOUT, 8)


def _build_trail(SA, SB, c):
    """Trail-cover table for core c.

    Returns (seq, pos) where seq is the row sequence (list of logical
    table rows, len <= LMAX) and pos[s] (s in [0,4096)) is the position
    of slot s's factor pair: seq[pos[s]] and seq[pos[s]+1] are the two
    factors of slot s.
    """
    A = SA[c * CORE_OUT:(c + 1) * CORE_OUT].reshape(-1)
    B = SB[c * CORE_OUT:(c + 1) * CORE_OUT].reshape(-1)

    from collections import defaultdict
    edge_pos = {}
    adj = defaultdict(list)       # node -> list of edge keys
    loops = []
    keys = []
    for a, b in zip(A.tolist(), B.tolist()):
        k = (a, b) if a <= b else (b, a)
        keys.append(k)
        if k not in edge_pos:
            edge_pos[k] = -1
            if a == b:
                loops.append(k)
            else:
                adj[k[0]].append(k)
                adj[k[1]].append(k)

    deg = {u: len(es) for u, es in adj.items()}
    used = set()
    seq = []

    def walk(start):
        # greedy trail walk, no splicing: consecutive nodes are always
        # genuine edges; leftover edges get their own trails later
        trail = [start]
        u = start
        while True:
            found = None
            while adj[u]:
                k = adj[u].pop()
                if k in used:
                    continue
                used.add(k)
                found = k
                break
            if found is None:
                break
            v = found[0] if found[1] == u else found[1]
            trail.append(v)
            u = v
        return trail

    # prefer odd-degree starts so each component needs fewest trails
    nodes = sorted(deg, key=lambda u: -(deg[u] % 2))
    for u in nodes:
        while any(k not in used for k in adj[u]):
            trail = walk(u)
            base = len(seq)
            seq.extend(trail)
            for i in range(len(trail) - 1):
                a, b = trail[i], trail[i + 1]
                k = (a, b) if a <= b else (b, a)
                if edge_pos[k] == -1:
                    edge_pos[k] = base + i
    for k in loops:
        edge_pos[k] = len(seq)
        seq.extend([k[0], k[0]])

    pos = np.array([edge_pos[k] for k in keys], dtype=np.int64)
    assert len(seq) <= LMAX, len(seq)
    return seq, pos


# per-chunk gather call plan: list of block spans [k0, k1) per chunk.
# 512-idx halves keep transfers flowing; the first chunk starts with
# 256-idx calls so the first transfer begins after minimal DGE, and the
# last chunk ends with 256-idx calls so the post-transfer DVE tail is
# only the final quarter's ops.
CALL_PLAN = ([[(0, 2), (2, 4), (4, 8)]]
             + [[(0, 4), (4, 8)]] * (NCHUNK - 2)
             + [[(0, 4), (4, 6), (6, 8)]])


def _wrap_core_idx(pos):
    """Wrap per-slot positions [4096] into the SWDGE index tensor
    [128, 4096//16] int16, ordered per CALL_PLAN.  Within a call over
    chunk cc blocks [k0,k1): element i fetches slot
    (cc*128 + i%128)*8 + (k0 + i//128)."""
    S = pos.reshape(CORE_OUT, 8)
    cols = []
    for cc, calls in enumerate(CALL_PLAN):
        blk = S[cc * 128:(cc + 1) * 128, :]      # [128 outs, 8 slots]
        for k0, k1 in calls:
            idx_call = blk[:, k0:k1].T.reshape(-1)
            w = idx_call.reshape(-1, 16).T.astype(np.int16)
            cols.append(np.tile(w, (8, 1)))
    return np.ascontiguousarray(np.concatenate(cols, axis=1))


# ----------------------------------------------------------------------------
# bass program (built once, cached)
# ----------------------------------------------------------------------------

_CACHED = {}


def _build_program():
    import concourse.bacc as bacc
    import concourse.mybir as mybir
    from concourse.tile import TileContext
    from concourse.ap import AP

    f32 = mybir.dt.float32
    bf16 = mybir.dt.bfloat16
    i16 = mybir.dt.int16

    nc = bacc.Bacc("TRN2", target_bir_lowering=False, debug=False)

    enc = nc.dram_tensor("enc", [LMAX, BATCH], bf16, kind="ExternalInput")
    idxe = nc.dram_tensor("idxe", [128, NCHUNK * GI // 16], i16,
                          kind="ExternalInput")
    out = nc.dram_tensor("out", [CORE_OUT, BATCH], f32, kind="ExternalOutput")

    E2 = 2 * BATCH   # gathered element: 2 consecutive rows = both factors

    with TileContext(nc) as tc:
        with tc.tile_pool(name="setup", bufs=1) as sp, \
             tc.tile_pool(name="gather", bufs=6) as gp, \
             tc.tile_pool(name="gatherq", bufs=2) as gq, \
             tc.tile_pool(name="mid", bufs=3) as mp, \
             tc.tile_pool(name="outp", bufs=2) as op:

            ia = sp.tile([128, NCHUNK * GI // 16], i16, tag="ia")
            nc.sync.dma_start(out=ia[:, :], in_=idxe[:, :])

            # overlapping-window view of enc: element p = rows [p, p+1]
            base = enc[:, :]
            enc_win = AP(tensor=base.tensor, offset=base.offset,
                         ap=[[BATCH, LMAX - 1], [1, E2]])

            icol = 0

            def gather(pool, nblk, tag):
                nonlocal icol
                g = pool.tile([128, nblk, E2], bf16, tag=tag)
                nidx = nblk * 128
                nc.gpsimd.dma_gather(
                    out_ap=g[:, :, :],
                    in_ap=enc_win,
                    idxs_ap=ia[:, icol:icol + nidx // 16],
                    num_idxs=nidx, num_idxs_reg=nidx,
                    elem_size=E2, elem_step=BATCH,
                    single_packet=False)
                icol += nidx // 16
                return g

            def half_tree(g):
                # block k of g = [factor_u | factor_v]; returns the
                # 4-slot half-tree result h2 = (s0*s1' ... ) [128,1,B]
                h0 = mp.tile([128, 4, BATCH], bf16, tag="h0h")
                nc.vector.tensor_mul(
                    h0[:, :, :], g[:, :, 0:BATCH], g[:, :, BATCH:E2])
                h1 = mp.tile([128, 2, BATCH], bf16, tag="h1h")
                nc.vector.tensor_add(
                    h1[:, :, :], h0[:, 0:4:2, :], h0[:, 1:4:2, :])
                h2 = mp.tile([128, 1, BATCH], bf16, tag="h2h")
                nc.vector.tensor_mul(
                    h2[:, :, :], h1[:, 0:1, :], h1[:, 1:2, :])
                return h2

            def emit_out(cc, h2a, h2b):
                h3 = op.tile([128, 1, BATCH], f32, tag="h3")
                nc.vector.tensor_add(
                    h3[:, :, :], h2a[:, :, :], h2b[:, :, :])
                nc.sync.dma_start(
                    out=out[cc * 128:(cc + 1) * 128, :]
                        .rearrange("(k p) f -> p k f", p=128),
                    in_=h3[:, :, :])

            def quarter_pair():
                # two 256-idx calls -> quarter trees -> h2 [128,1,B]
                ps = []
                for _ in range(2):
                    g = gather(gq, 2, "gq")
                    q = mp.tile([128, 2, BATCH], bf16, tag="h0q")
                    nc.vector.tensor_mul(
                        q[:, :, :], g[:, :, 0:BATCH], g[:, :, BATCH:E2])
                    p = mp.tile([128, 1, BATCH], bf16, tag="h1q")
                    nc.vector.tensor_add(
                        p[:, :, :], q[:, 0:1, :], q[:, 1:2, :])
                    ps.append(p)
                h2 = mp.tile([128, 1, BATCH], bf16, tag="h2h")
                nc.vector.tensor_mul(h2[:, :, :], ps[0][:, :, :],
                                     ps[1][:, :, :])
                return h2

            # chunk 0: 256+256 first (earliest transfer start), then 512
            h2a = quarter_pair()
            h2b = half_tree(gather(gp, 4, "gh"))
            emit_out(0, h2a, h2b)

            # chunks 1..NCHUNK-2: two 512-idx halves each
            for cc in range(1, NCHUNK - 1):
                h2a = half_tree(gather(gp, 4, "gh"))
                h2b = half_tree(gather(gp, 4, "gh"))
                emit_out(cc, h2a, h2b)

            # last chunk: 512 + 256 + 256 so the post-transfer DVE tail
            # is only the last quarter's ops
            h2a = half_tree(gather(gp, 4, "gh"))
            h2b = quarter_pair()
            emit_out(NCHUNK - 1, h2a, h2b)

    nc.compile()
    return nc


def _get_program():
    if "nc" not in _CACHED:
        _CACHED["nc"] = _build_program()
    return _CACHED["nc"]


# ----------------------------------------------------------------------------
# public entry point
# ----------------------------------------------------------------------------

def kernel(x, idx0, idx1, idx2, idx3, _trace=False, _trace_kwargs=None):
    import ml_dtypes
    from concourse.bass_utils import run_bass_kernel_spmd

    x = np.asarray(x, dtype=np.float32)
    enc_np = np.empty((TABLE_ROWS, BATCH), np.float32)
    enc_np[0:N_VARS] = x
    enc_np[N_VARS:2 * N_VARS] = 1.0 - x
    enc_np[2 * N_VARS] = 0.0
    enc_np[2 * N_VARS + 1] = 1.0
    enc_bf = enc_np.astype(ml_dtypes.bfloat16)
    S_A, S_B = _compose_indices(
        np.asarray(idx0), np.asarray(idx1), np.asarray(idx2), np.asarray(idx3))

    nc = _get_program()
    in_maps = []
    for c in range(NCORES):
        seq, pos = _build_trail(S_A, S_B, c)
        table = np.zeros((LMAX, BATCH), ml_dtypes.bfloat16)
        table[:len(seq)] = enc_bf[np.asarray(seq, dtype=np.int64)]
        in_maps.append({
            "enc": table,
            "idxe": _wrap_core_idx(pos),
        })

    kwargs = {}
    if _trace:
        kwargs["trace"] = True
        if _trace_kwargs:
            kwargs.update(_trace_kwargs)
    res = run_bass_kernel_spmd(nc, in_maps, core_ids=list(range(NCORES)), **kwargs)
    outs = [res.results[c]["out"] for c in range(NCORES)]
    full = np.concatenate(outs, axis=0)
    if _trace:
        kernel.last_exec_time_ns = res.exec_time_ns
        kernel.last_profile = res.profile_json
    return full
